# revision 88
# baseline (speedup 1.0000x reference)
"""Causal self-attention (B=8, T=1024, C=768, NH=12) on 8 TRN2 NeuronCores.

Sharding: pure data parallel - one batch element per core, no collectives.

Host side: x is transposed to xT [C, T] and cast to bf16 (with w_attn, w_proj);
biases stay fp32.

Per-core kernel (Bass/Tile):
  1. xT DMA'd straight into SBUF [128, 6, 1024] (no on-device transpose).
  2. QKV: w-stationary bf16 matmuls. q/k tiles are written PSUM->SBUF as
     fp8e4 (tensor_scalar add of per-partition bias), then DMA-regrouped
     SBUF->SBUF into [32, head, 2, T] so each head's 64 contraction channels
     sit as two 32-partition groups -> scores run as a single fp8 DoubleRow
     matmul per (head, tk) at 0.5 cycles/row. v rows land row-major with an
     appended ones column per head (vaug [128, 8, 12*65]).
  3. Attention per head pair, tq-block b of 512, causally trimmed tk tiles:
       pst[128, 2, nn] = DoubleRow scores (both heads)
       ut = exp(0.125 * pst) (ACT, bf16; no max-subtraction - scores bounded)
       diagonal 128 cols get a multiplicative 0/1 mask on GPSIMD (idle engine)
       PV is U-stationary: pys[tq=128, c, h, 65] += ut_chunk.T @ vaug_tile
         (output free size 65 instead of 512 -> 2x fewer PE cycles; the ones
         column makes row 64 the softmax denominator, now a per-partition
         scalar)
       normalization = single DVE divide with stride-0 broadcast of col 64
       yn -> PE transpose (2 heads packed [tq, 128]) -> yT [128, hp, T]
  4. b=0 attention for all pairs first, then proj m-tiles 0..3 overlap the
     b=1 attention round; out = yT.T @ w_proj + b_proj -> [T, C] fp32.

Matmul operand dtypes: bf16 (full rate) for QKV/PV/proj, fp8e4 DoubleRow
(double rate) for scores - q/k quantization to fp8 measured at 7e-3 rel err
end to end (v/U/x/y quantization fails the 2e-2 gate, so those stay bf16).
PSUM accumulation is fp32. SBUF pools are never recycled for DMA-written
tiles; PSUM pools recycle (compute-only accessors).
"""

import numpy as np
import ml_dtypes

import concourse.bass as bass
import concourse.bacc as bacc
import concourse.tile as tile
from concourse import mybir
from concourse.bass_utils import run_bass_kernel_spmd

B, T, C = 8, 1024, 768
NH, HD = 12, 64
P = 128
KC = C // P          # 6 k-tiles over C
KT = T // P          # 8 tiles over T
NHP = NH // 2        # 6 head pairs
TQB = 512            # tq block
NB = T // TQB        # 2 tq blocks
NCH = TQB // P       # 4 tq chunks of 128 per block
VW = HD + 1          # 65: v columns + ones column per head

F32 = mybir.dt.float32
BF16 = mybir.dt.bfloat16
FP8 = mybir.dt.float8e4
FT = mybir.ActivationFunctionType
DR = mybir.MatmulPerfMode.DoubleRow


def build_program():
    nc = bacc.Bacc("TRN2", target_bir_lowering=False, debug=False)
    # x pre-transposed, pair-chunked: [q, p, k, tt2, f] = xT[k*128+p, q*256+tt2*128+f]
    xtb_d = nc.dram_tensor("xtb", [KT // 2, P, KC, 2, P], BF16, kind="ExternalInput").ap()
    # fp8 copy of xT for the q/k projection, DoubleRow pair layout:
    # [q, p, kk, i, tt2, f] = xT[(2kk+i)*128+p, q*256+tt2*128+f]
    xt8_d = nc.dram_tensor(
        "xt8", [KT // 2, P, KC // 2, 2, 2, P], FP8, kind="ExternalInput"
    ).ap()
    # q/k weights (x32, fp8) in consumption order j -> m = (j%2)*KC + j//2:
    # [j, p, kk, i, n] = 32 * w_attn[(2kk+i)*128+p, m*128+n]
    waqk_d = nc.dram_tensor(
        "waqk", [2 * KC, P, KC // 2, 2, P], FP8, kind="ExternalInput"
    ).ap()
    # v-part and proj weights in [p, k, n] device layout
    wav_d = nc.dram_tensor("wav", [P, KC, C], BF16, kind="ExternalInput").ap()
    wpt_d = nc.dram_tensor("wpt", [P, KC, C], BF16, kind="ExternalInput").ap()
    ba_d = nc.dram_tensor("b_attn", [3 * C], F32, kind="ExternalInput").ap()
    bp_d = nc.dram_tensor("b_proj", [C], F32, kind="ExternalInput").ap()
    out_d = nc.dram_tensor("out", [T, C], BF16, kind="ExternalOutput").ap()

    from contextlib import ExitStack

    with tile.TileContext(nc) as tc:
        with ExitStack() as ctx:
            _body(ctx, tc, xtb_d, xt8_d, waqk_d, wav_d, wpt_d, ba_d, bp_d, out_d)
    nc.compile()
    return nc


def _body(ctx, tc, xtb_d, xt8_d, waqk_d, wav_d, wpt_d, ba_d, bp_d, out_d):
    nc = tc.nc

    const = ctx.enter_context(tc.tile_pool(name="const", bufs=1))
    persist = ctx.enter_context(tc.tile_pool(name="persist", bufs=1))
    wqk_pool = ctx.enter_context(tc.tile_pool(name="wqk", bufs=12))
    upool = ctx.enter_context(tc.tile_pool(name="upool", bufs=18))
    snorm = ctx.enter_context(tc.tile_pool(name="snorm", bufs=4))

    # constants ------------------------------------------------------------
    ident = const.tile([P, P], BF16)
    nc.gpsimd.memset(ident, 0.0)
    nc.gpsimd.affine_select(
        out=ident, in_=ident, compare_op=mybir.AluOpType.not_equal,
        fill=1.0, base=0, pattern=[[-1, P]], channel_multiplier=1,
    )
    # multiplicative causal mask: 1 where tk <= tq else 0 (applied post-exp)
    tri01 = const.tile([P, P], BF16)
    nc.gpsimd.memset(tri01, 1.0)
    nc.gpsimd.affine_select(
        out=tri01, in_=tri01, compare_op=mybir.AluOpType.is_ge,
        fill=0.0, base=0, pattern=[[1, P]], channel_multiplier=-1,
    )
    # b_attn for q/k as per-partition scalars: [p, m] with b[128m + p]
    battn_pm = const.tile([P, 2 * KC], F32)

    # b_attn v-part / b_proj broadcast along partitions: [128, 768]
    def _pbcast(src):
        return bass.AP(tensor=src.tensor, offset=src.offset, ap=[[0, P]] + list(src.ap))

    bv_b = const.tile([P, C], F32)
    bp_b = const.tile([P, C], F32)
    # b_proj folded into the proj matmul as a K=1 accumulation step
    ones_row = const.tile([1, P], BF16)
    nc.vector.memset(ones_row, 1.0)
    bp_row = const.tile([1, C], BF16)
    bv_row = const.tile([1, C], BF16)

    # persistent SBUF tensors ---------------------------------------------
    xT = persist.tile([P, KC, T], BF16)           # [128, 6, 1024] 1.5 MB
    xt8 = persist.tile([P, KC // 2, 2, T], FP8)   # fp8 x for q/k projection
    # q/k channels in fp8 with a zeros column per m-tile: scores run as
    # K=64 DoubleRow matmuls whose second k-slot reads the zeros, so no
    # partition-regroup DMA is ever needed
    qkz = persist.tile([P, 2 * KC, 2, T], FP8)
    vaug = persist.tile([P, KT, NH * VW], BF16)   # [128, 8, 780]
    # per-(b, pair) yT tensors: DMA-transpose writes are tracked at tile
    # granularity, so a proj k-step must only depend on ITS pair's transpose,
    # not whichever transpose happened to write the shared tile last
    yTp = [
        [persist.tile([P, TQB], BF16, name=f"yT{b}_{hp}") for hp in range(NHP)]
        for b in range(NB)
    ]
    wv_sb = persist.tile([P, KC, C], BF16)
    wp_sb = persist.tile([P, KC, C], BF16)
    ot = persist.tile([P, KT, C], BF16)           # [128, 8, 768]

    # startup order: first x pair-chunk, v weights in halves, bias
    # broadcast, remaining x pair-chunks
    nc.sync.dma_start(out=xT[:, :, 0 : 2 * P], in_=xtb_d[0])
    nc.sync.dma_start(out=wv_sb[:, 0:3, :], in_=wav_d[:, 0:3, :])
    nc.sync.dma_start(out=wv_sb[:, 3:6, :], in_=wav_d[:, 3:6, :])
    nc.sync.dma_start(out=bv_b, in_=_pbcast(ba_d[2 * C : 3 * C]))
    # xt8 before the later xtb chunks: the q/k projection (and with it the
    # whole attention pipeline) unblocks as early as possible
    for q in range(KT // 2):
        nc.sync.dma_start(
            out=xt8[:, :, :, q * 2 * P : (q + 1) * 2 * P],
            in_=xt8_d[q],
        )
    for q in range(1, KT // 2):
        nc.sync.dma_start(
            out=xT[:, :, q * 2 * P : (q + 1) * 2 * P],
            in_=xtb_d[q],
        )
    nc.sync.dma_start(
        out=battn_pm, in_=ba_d[0 : 2 * C].rearrange("(m p) -> p m", p=P)
    )
    # q/k land x32 in qkz, so the bias must be pre-scaled to match
    battn32 = const.tile([P, 2 * KC], F32)
    nc.vector.tensor_scalar(
        out=battn32, in0=battn_pm, scalar1=float(32), scalar2=None,
        op0=mybir.AluOpType.mult,
    )
    # zero the i=1 k-slots (Pool is idle; per-m chunks ordered so pair 0's
    # tiles are ready first)
    for m in [0, KC, 1, KC + 1, 2, KC + 2, 3, KC + 3, 4, KC + 4, 5, KC + 5]:
        nc.gpsimd.memset(qkz[:, m, 1, :], 0.0)

    with (
        tc.tile_pool(name="mmpsum", bufs=1, space="PSUM") as mmpsum,
        tc.tile_pool(name="spsum", bufs=2, space="PSUM") as spsum,
        tc.tile_pool(name="ypsum", bufs=1, space="PSUM") as ypsum,
    ):
        # preload all q/k weight tiles up front (HWDGE is the serial
        # resource; issuing these early keeps the prefetched qk matmuls from
        # ever waiting on a weight DMA mid-round)
        wqk_all = persist.tile([P, 2 * KC, KC // 2, 2, P], FP8, name="wqk_all")
        order = [0, KC, 1, KC + 1, 2, KC + 2, 3, KC + 3, 4, KC + 4, 5, KC + 5]
        wts = {m: wqk_all[:, j] for j, m in enumerate(order)}
        for j0 in (0, 4, 8):
            nc.sync.dma_start(
                out=wqk_all[:, j0 : j0 + 4],
                in_=waqk_d.rearrange("j p kk i n -> p j kk i n")[:, j0 : j0 + 4],
            )


        def qk_tile(m):
            """QKV matmul for q/k column tile m -> fp8 qkz (per-n copies)."""
            wt = wts[m]
            # (b) alternate the psum slot between the mm and pst tags so the
            # second tile of a prefetch pair never waits on the first tile's
            # PSUM->SBUF copies
            if m < KC:
                ps = mmpsum.tile([P, NB, TQB], F32, name=f"qkps{m}", tag="mm")
            else:
                ps = spsum.tile([P, NB, TQB], F32, name=f"qkps{m}", tag="pst")
            for n in range(NB):
                for kk in range(KC // 2):
                    nc.tensor.matmul(
                        ps[:, n, :],
                        wt[:, kk, :, :],
                        xt8[:, kk, :, n * TQB : (n + 1) * TQB],
                        start=(kk == 0),
                        stop=(kk == KC // 2 - 1),
                        perf_mode=DR,
                    )
            # per-n copies split across ACT and DVE so the next tile's WAR on
            # the single mm psum slot clears in one copy-time, not two
            with nc.allow_low_precision(reason="fp8 scores operands"):
                nc.vector.tensor_scalar(
                    out=qkz[:, m, 0, 0:TQB],
                    in0=ps[:, 0, :],
                    scalar1=battn32[:, m : m + 1],
                    scalar2=None,
                    op0=mybir.AluOpType.add,
                )
                nc.vector.tensor_scalar(
                    out=qkz[:, m, 0, TQB : 2 * TQB],
                    in0=ps[:, 1, :],
                    scalar1=battn32[:, m : m + 1],
                    scalar2=None,
                    op0=mybir.AluOpType.add,
                )

        # ---- v rows (+bias), with interleaved ones cols ------------------
        vhe = vaug[:, :, :].rearrange("p t (h e) -> p t h e", e=VW)
        nc.vector.memset(vhe[:, :, :, HD : HD + 1], 1.0)
        def v_tile(tt):
            vpool = mmpsum if tt % 2 == 0 else spsum
            vtag = "mm" if tt % 2 == 0 else "pst"
            ps = vpool.tile([P, NB, TQB], F32, name=f"vps{tt}", tag=vtag)
            for n in range(NB):
                nsz = min(TQB, C - n * TQB)  # 512, 256
                for k in range(KC):
                    nc.tensor.matmul(
                        ps[:, n, :nsz],
                        xT[:, k, tt * P : (tt + 1) * P],
                        wv_sb[:, k, n * TQB : n * TQB + nsz],
                        start=(k == 0),
                        stop=(k == KC - 1),
                    )
            for n in range(NB):
                nsz = min(TQB, C - n * TQB)
                nh0 = n * TQB // HD
                nh = nsz // HD
                nc.vector.tensor_tensor(
                    out=vhe[:, tt, nh0 : nh0 + nh, 0:HD],
                    in0=ps[:, n, :nsz].rearrange("p (h e) -> p h e", e=HD),
                    in1=bv_b[:, n * TQB : n * TQB + nsz].rearrange(
                        "p (h e) -> p h e", e=HD
                    ),
                    op=mybir.AluOpType.add,
                )

        for tt in range(NCH):
            v_tile(tt)
        qk_tile(0)
        qk_tile(KC)


        def attn_A(hp, b):
            """Phase A: scores (fp8 DoubleRow) + exp + diag mask over all tk.

            Emitted one block AHEAD of the matching phase B so the next
            block's exps are already in the ACT queue while this block's PV
            runs - the activation engine never waits on the PE stream.
            """
            ntk = NCH * (b + 1)
            uts = []
            for tk in range(ntk):
                diag = (tk // NCH) == b
                off = tk * P - b * TQB if diag else 0
                nn = TQB - off
                pst = spsum.tile([P, 2, TQB], F32, name="pst", tag="pst")
                ut = upool.tile([P, 2, TQB], BF16, name="ut")
                uts.append(ut)
                for h2 in range(2):
                    lo, hi = 64 * h2, 64 * h2 + 64
                    nc.tensor.matmul(
                        pst[:, h2, off:TQB],
                        qkz[lo:hi, KC + hp, :, tk * P : (tk + 1) * P],
                        qkz[lo:hi, hp, :, b * TQB + off : (b + 1) * TQB],
                        start=True,
                        stop=True,
                        perf_mode=DR,
                    )
                nc.scalar.activation(
                    out=ut[:, :, 0:nn],
                    in_=pst[:, :, off:TQB],
                    func=FT.Exp,
                    scale=0.125 / (32.0 * 32.0),
                )
                if diag:
                    nc.vector.tensor_tensor(
                        out=ut[:, :, 0:P],
                        in0=ut[:, :, 0:P],
                        in1=tri01[:, None, :].to_broadcast([P, 2, P]),
                        op=mybir.AluOpType.mult,
                    )
            return (hp, b, uts)

        def attn_B(state, split_norm=False):
            """Phase B: U-stationary PV, normalization, transpose to yT."""
            hp, b, uts = state
            pys = ypsum.tile([P, NCH, 2, P], F32, name=f"py{hp}_{b}", tag="pys")
            for c in range(NCH):
                last = NCH * b + c
                for h2 in range(2):
                    h = 2 * hp + h2
                    for tk in range(last + 1):
                        c0 = tk - NCH * b if tk >= NCH * b else 0
                        nc.tensor.matmul(
                            pys[:, c, h2, 0:VW],
                            uts[tk][:, h2, (c - c0) * P : (c - c0 + 1) * P],
                            vaug[:, tk, h * VW : (h + 1) * VW],
                            start=(tk == 0),
                            stop=(tk == last),
                        )
            del uts
            # normalize: y / rowsum (col 64); walrus allows only one PSUM
            # input per op, so stage the 8 denominators in SBUF first
            yn = snorm.tile([P, NCH, 2, HD], BF16, name="yn")
            ysum = snorm.tile([P, NCH, 2, 1], F32, name="ysum", tag="ysum")
            halves = [(0, NCH)] if not split_norm else [(0, 2), (2, NCH)]
            for c0h, c1h in halves:
                with nc.allow_low_precision(reason="bf16 softmax normalization"):
                    nc.vector.reciprocal(
                        out=ysum[:, c0h:c1h], in_=pys[:, c0h:c1h, :, HD : HD + 1]
                    )
                    nc.vector.tensor_tensor(
                        out=yn[:, c0h:c1h],
                        in0=pys[:, c0h:c1h, :, 0:HD],
                        in1=ysum[:, c0h:c1h].to_broadcast([P, c1h - c0h, 2, HD]),
                        op=mybir.AluOpType.mult,
                    )
                # blocked transpose [tq, (c,h,hd)] -> [(h,hd), (c,tq)] in
                # one xbar DMA: out[ch, c, t] = yn[t, c*128 + ch]
                nc.sync.dma_start_transpose(
                    out=yTp[b][hp][:, c0h * P : c1h * P].rearrange(
                        "p (c f) -> p c f", c=c1h - c0h
                    ),
                    in_=yn[:, c0h:c1h].rearrange("p c h e -> p (c h e)"),
                )

        def proj_tile(m):
            """out[t-tile m] = yT.T @ w_proj + b_proj (bias via K=1 matmul)."""
            # in-round proj tiles must not touch the pst tag (it is the
            # attention score double-buffer); only the tail tiles (m>=4, all
            # attention done) may alternate into it
            if m < 4 or m % 2 == 0:
                ps = mmpsum.tile([P, NB, TQB], F32, name=f"ops{m}", tag="mm")
            else:
                ps = spsum.tile([P, NB, TQB], F32, name=f"ops{m}", tag="pst")
            otr = out_d.rearrange("(t p) c -> p t c", p=P)
            if m < KT - 2:
                for k in range(KC):
                    for n in range(NB):
                        nsz = min(TQB, C - n * TQB)
                        nc.tensor.matmul(
                            ps[:, n, :nsz],
                            yTp[m // NCH][k][:, (m % NCH) * P : (m % NCH + 1) * P],
                            wp_sb[:, k, n * TQB : n * TQB + nsz],
                            start=(k == 0),
                            stop=(k == KC - 1),
                        )
                for n in range(NB):
                    nsz = min(TQB, C - n * TQB)
                    with nc.allow_low_precision(reason="bf16 output"):
                        nc.vector.tensor_tensor(
                            out=ot[:, m, n * TQB : n * TQB + nsz],
                            in0=ps[:, n, :nsz],
                            in1=bp_b[:, n * TQB : n * TQB + nsz],
                            op=mybir.AluOpType.add,
                        )
                    nc.sync.dma_start(
                        out=otr[:, m, n * TQB : n * TQB + nsz],
                        in_=ot[:, m, n * TQB : n * TQB + nsz],
                    )
            else:
                # last two tiles: fully per-n staggered exit (matmuls, copy,
                # DMA per half) to shorten the post-PE tail
                for n in range(NB):
                    nsz = min(TQB, C - n * TQB)
                    for k in range(KC):
                        nc.tensor.matmul(
                            ps[:, n, :nsz],
                            yTp[m // NCH][k][:, (m % NCH) * P : (m % NCH + 1) * P],
                            wp_sb[:, k, n * TQB : n * TQB + nsz],
                            start=(k == 0),
                            stop=(k == KC - 1),
                        )
                    with nc.allow_low_precision(reason="bf16 output"):
                        nc.vector.tensor_tensor(
                            out=ot[:, m, n * TQB : n * TQB + nsz],
                            in0=ps[:, n, :nsz],
                            in1=bp_b[:, n * TQB : n * TQB + nsz],
                            op=mybir.AluOpType.add,
                        )
                    nc.sync.dma_start(
                        out=otr[:, m, n * TQB : n * TQB + nsz],
                        in_=ot[:, m, n * TQB : n * TQB + nsz],
                    )

        # b=0 attention for all pairs (prefetching remaining qk tiles), then
        # proj for the first 512 rows interleaves with the b=1 round.
        # (proj-only loads deferred here, past the startup DMA crunch)
        nc.sync.dma_start(out=wp_sb[:, 0:3, :], in_=wpt_d[:, 0:3, :])
        nc.sync.dma_start(out=wp_sb[:, 3:6, :], in_=wpt_d[:, 3:6, :])
        nc.sync.dma_start(out=bp_b, in_=_pbcast(bp_d))
        with nc.allow_low_precision(reason="bf16 bias row for K=1 matmul"):
            nc.vector.tensor_copy(out=bp_row, in_=bp_b[0:1, :])
        # globally ACT-paced interleave with one-block software pipeline:
        # phase A of block i+1 is emitted before phase B of block i, so the
        # ACT queue always holds the next exps while the PE runs PV/fillers
        seq = [
            ((0, 0), (1, KC + 1), None),
            ((1, 0), (2, KC + 2), None),
            ((2, 0), (3, KC + 3), None),
            ((3, 0), (4, KC + 4), None),
            ((4, 0), (5, KC + 5), None),
            ((5, 0), None, None),
            ((0, 1), None, None),
            ((1, 1), None, None),
            ((2, 1), None, (0,)),
            ((3, 1), None, (1,)),
            ((4, 1), None, (2,)),
            ((5, 1), None, (3,)),
        ]
        pending = None
        for bi, ((hp, b), pf, projs) in enumerate(seq):
            if pf is not None:
                qk_tile(pf[0])
                qk_tile(pf[1])
            st = attn_A(hp, b)
            if bi == 6:
                # late v tiles emitted under the shadow of (0,1)'s queued
                # exps; their first consumer is B(0,1) one block later
                for tt in range(NCH, KT):
                    v_tile(tt)
            if pending is not None:
                attn_B(pending)
            if projs is not None:
                for m in projs:
                    proj_tile(m)
            pending = st
        attn_B(pending, split_norm=True)
        for m in (4, 5, 6, 7):
            proj_tile(m)


_prog_cache = {}


def _get_program():
    if "nc" not in _prog_cache:
        _prog_cache["nc"] = build_program()
    return _prog_cache["nc"]


def kernel(x, w_attn, b_attn, w_proj, b_proj, _trace=False):
    nc = _get_program()
    bf = ml_dtypes.bfloat16
    # [B, C, T] -> [B, q, p, k, tt2, f] with c = k*128+p, t = q*256+tt2*128+f
    xtb = np.ascontiguousarray(
        np.asarray(x, dtype=np.float32)
        .transpose(0, 2, 1)
        .reshape(B, KC, P, KT // 2, 2, P)
        .transpose(0, 3, 2, 1, 4, 5)
        .astype(bf)
    )
    # [C, 2C] -> [m, p, k, n]
    f8 = ml_dtypes.float8_e4m3
    _order = [0, KC, 1, KC + 1, 2, KC + 2, 3, KC + 3, 4, KC + 4, 5, KC + 5]
    # [j, p, kk, i, n] = 32 * w[(2kk+i)*128+p, m*128+n], fp8
    waqk = np.ascontiguousarray(
        (np.asarray(w_attn[:, : 2 * C], dtype=np.float32) * 32.0)
        .reshape(KC // 2, 2, P, 2 * KC, P)
        .transpose(3, 2, 0, 1, 4)[_order]
        .astype(f8)
    )
    # [q, p, kk, i, tt2, f] = xT[(2kk+i)*128+p, q*256+tt2*128+f], fp8
    xt8 = np.ascontiguousarray(
        np.asarray(x, dtype=np.float32)
        .transpose(0, 2, 1)
        .reshape(B, KC // 2, 2, P, KT // 2, 2, P)
        .transpose(0, 4, 3, 1, 2, 5, 6)
        .astype(f8)
    )
    wav = np.ascontiguousarray(
        np.asarray(w_attn[:, 2 * C :], dtype=np.float32)
        .reshape(KC, P, C)
        .transpose(1, 0, 2)
        .astype(bf)
    )
    wpt = np.ascontiguousarray(
        np.asarray(w_proj, dtype=np.float32).reshape(KC, P, C).transpose(1, 0, 2).astype(bf)
    )
    b_attn = np.ascontiguousarray(np.asarray(b_attn, dtype=np.float32))
    b_proj = np.ascontiguousarray(np.asarray(b_proj, dtype=np.float32))
    in_maps = [
        {
            "xtb": xtb[b],
            "xt8": xt8[b],
            "waqk": waqk,
            "wav": wav,
            "wpt": wpt,
            "b_attn": b_attn,
            "b_proj": b_proj,
        }
        for b in range(B)
    ]
    res = run_bass_kernel_spmd(nc, in_maps, list(range(B)), trace=_trace)
    out = np.stack(
        [np.asarray(res.results[i]["out"], dtype=np.float32) for i in range(B)], axis=0
    )
    if _trace:
        kernel.last_results = res
    return out
